# revision 7
# baseline (speedup 1.0000x reference)
"""CRF-RNN (crfasrnn) Bass kernel for 8 Trainium2 NeuronCores — fp8 edition.

N=8192 voxels, L=21 labels. Each core owns a 1024-voxel column block of the
two NxN Gaussian kernels K_sp/K_bi. BOTH kernels are built once into SBUF as
fp8e4 bytes (64KB/partition each) — vs the bf16 baseline that had to rebuild
K_bi every mean-field iteration. The filtering matmul runs in fp8 DoubleRow
perf mode (2 contraction j-tiles per pass): 2x the bf16 byte rate, so the 5
iterations cost half the PE stream time AND skip per-iteration gram+exp.
LDWEIGHTS fully pipelines under the matmul stream (background weight
buffer), so the filter runs at the 216ns/512-col MM floor.

q precision is restored with a two-limb fp8 representation: q ~ hi + lo/64,
hi = fp8(q), lo = fp8(64*(q - hi)) — net error ~bf16-level. Both limbs ride
in one DoubleRow stationary set (43 rows: 21 hi | ones | 21 lo), so K
streams ONCE for numerator, normalizer, and both limbs. The 1/64 unscale
AND the normalizer extraction are folded into the 21x21 label matmul: the
w-stack is [W; e-row; W/64] with an extra indicator column that routes the
ones-row sum (n) into output column 21 — no separate 1-column matmuls.

K bytes: the spatial kernel is encoded by a single DVE op per j-tile —
byte = round((gram + sqj')*11.5416) saturated to [0,255] and bitcast as
fp8e4 is exp(gram+sqj) to within a half-byte step (fp8's bit pattern is a
piecewise-log code; HW-verified round-to-nearest + saturation). The
bilateral kernel (near-diagonal, 5x weight — precision-sensitive; sim: DVE
encode there doubles final error) uses exact ACT exp with fp8 output,
which also balances build work across DVE and ACT. Mixing encoders within
one kernel is the worst of both (inconsistent per-row bias — sim'd).

Scheduling: iteration-0's softmax is emitted BEFORE the build so its ACT
ops aren't queued behind 64 bilateral exps, and iteration-0's filter pairs
are interleaved into the build loop right after their two j-tiles are
encoded — the filter rides the build tail instead of serializing. Each
iteration's S accumulates in separate h-half PSUM tiles so the label phase
for i-tiles 0-3 starts while the h1 filter pass is still streaming. Per
iteration the cores all-gather their (1024,48)-byte q-limb slices in 4
chunks issued as soon as each pair of i-tiles softmaxes; the next filter
consumes pairs in chunk order. A 16-byte warm-up AllGather during the
build absorbs the first collective's ~20us channel-setup cost.

DoubleRow constraints (BIR verifier): the pair dim of both APs must be
dim1 with num=2 and a 16-byte-multiple stride — hence qT's inner stride of
48 bytes and K stored [128, NB, R]. Stationary free size 2x43=86 <= 128.
"""

import sys

sys.path.insert(0, "/opt/trn_rl_repo")

import numpy as np

NUM_CORES = 8
GAMMA, ALPHA, BETA = 3.0, 160.0, 3.0
NUM_ITERS = 5
L = 21
D, H, W = 8, 32, 32
N = D * H * W          # 8192
R = N // NUM_CORES     # 1024 columns per core
NB = N // 128          # 64 j tiles
NP = NB // 2           # 32 j-tile pairs (DoubleRow)
RT = R // 128          # 8 i tiles per core
FSP = 14               # 3 spatial features x4 hi/lo cross limbs + 2 sq rows
FBI = 26               # 6 bilateral features x4 hi/lo cross limbs + 2 sq rows
LW = 43                # stationary rows: 21 q_hi | ones@21 | 21 q_lo
QS = 48                # qT inner stride (multiple of 16 for DoubleRow)
LO = 22                # q_lo column offset in qT
GC = 4                 # gather chunks per iteration
BSCALE = 11.5416       # 8/ln2: fp8e4 byte steps per unit of ln K
BOFF = 4.93            # byte-encode offset (sim-calibrated)

_CACHE = {}


def _build(num_iters=NUM_ITERS):
    key = ("fp8v2", num_iters)
    if key in _CACHE:
        return _CACHE[key]

    import concourse.bacc as bacc
    import concourse.mybir as mybir
    import concourse.tile as tile

    f32 = mybir.dt.float32
    u8 = mybir.dt.uint8
    f8 = mybir.dt.float8e4
    bf16 = mybir.dt.bfloat16
    EXP = mybir.ActivationFunctionType.Exp
    MUL = mybir.AluOpType.mult
    ADD = mybir.AluOpType.add
    SUB = mybir.AluOpType.subtract
    DR = mybir.MatmulPerfMode.DoubleRow

    nc = bacc.Bacc(
        "TRN2", target_bir_lowering=False, debug=False, num_devices=NUM_CORES,
    )

    usp = nc.dram_tensor("usp", [FSP, N], bf16, kind="ExternalInput").ap()
    vsp = nc.dram_tensor("vsp", [FSP, R], bf16, kind="ExternalInput").ap()
    ubi = nc.dram_tensor("ubi", [FBI, N], bf16, kind="ExternalInput").ap()
    vbi = nc.dram_tensor("vbi", [FBI, R], bf16, kind="ExternalInput").ap()
    sqjd = nc.dram_tensor("sqjd", [128, NB], f32, kind="ExternalInput").ap()
    sqja = nc.dram_tensor("sqja", [128, NB], f32, kind="ExternalInput").ap()
    unt = nc.dram_tensor("unt", [R, L], f32, kind="ExternalInput").ap()
    lgt = nc.dram_tensor("lgt", [N, L], f32, kind="ExternalInput").ap()
    wsp = nc.dram_tensor("wsp", [LW, L + 1], f32, kind="ExternalInput").ap()
    wbi = nc.dram_tensor("wbi", [LW, L + 1], f32, kind="ExternalInput").ap()
    outq = nc.dram_tensor("outq", [R, L], f32, kind="ExternalOutput").ap()

    qsl = nc.dram_tensor("qsl", [R, QS], u8).ap()
    qfull = nc.dram_tensor(
        "qfull", [GC, NUM_CORES, R // GC, QS], u8, addr_space="Shared"
    ).ap()
    wrm_i = nc.dram_tensor("wrm_i", [1, 16], u8).ap()
    wrm_o = nc.dram_tensor("wrm_o", [NUM_CORES, 16], u8, addr_space="Shared").ap()

    with tile.TileContext(nc) as tc:
        with (
            tc.tile_pool(name="const", bufs=1) as cpool,
            tc.tile_pool(name="ssb", bufs=2) as wpool,
            tc.tile_pool(name="ustream", bufs=4) as upool,
            tc.tile_pool(name="small", bufs=4) as spool,
            tc.tile_pool(name="gps", bufs=2, space="PSUM") as gpool,
            tc.tile_pool(name="sps", bufs=1, space="PSUM") as s_pool,
        ):
            vsp_sb = cpool.tile([FSP, R], bf16)
            nc.sync.dma_start(vsp_sb[:], vsp)
            vbi_sb = cpool.tile([FBI, R], bf16)
            nc.sync.dma_start(vbi_sb[:], vbi)
            sqjd_sb = cpool.tile([128, NB], f32)
            nc.sync.dma_start(sqjd_sb[:], sqjd)
            sqja_sb = cpool.tile([128, NB], f32)
            nc.sync.dma_start(sqja_sb[:], sqja)
            wsp_sb = cpool.tile([LW, L + 1], f32)
            nc.sync.dma_start(wsp_sb[:], wsp)
            wbi_sb = cpool.tile([LW, L + 1], f32)
            nc.sync.dma_start(wbi_sb[:], wbi)
            unt_sb = cpool.tile([128, RT, L], f32)
            nc.sync.dma_start(unt_sb[:], unt.rearrange("(t p) l -> p t l", p=128))

            # q stationary: [hi 0:21 | ones 21 | lo 22:43 | pad], fp8 bytes
            qT = cpool.tile([128, NB, QS], u8)
            nc.vector.memset(qT[:], 0)
            nc.vector.memset(qT[:, :, 21], 0x38)      # fp8e4 1.0
            # gather staging, same column layout
            stg = cpool.tile([128, RT, QS], u8)
            nc.vector.memset(stg[:], 0)
            nc.vector.memset(stg[:, :, 21], 0x38)

            ksp_st = cpool.tile([128, NB, R], u8)
            kbi_st = cpool.tile([128, NB, R], u8)

            # warm the collective channels while the build runs
            wz = spool.tile([1, 16], u8, tag="wz")
            nc.vector.memset(wz[:], 0)
            nc.sync.dma_start(wrm_i, wz[:])
            nc.gpsimd.collective_compute(
                "AllGather", mybir.AluOpType.bypass,
                replica_groups=[list(range(NUM_CORES))],
                ins=[wrm_i.opt()], outs=[wrm_o.opt()],
            )

            # ---- iteration-0 softmax of the full logits (emitted first so
            # ACT/DVE run it before the build conversions) ----
            lg_sb = cpool.tile([128, NB, L], f32)
            nc.sync.dma_start(lg_sb[:], lgt.rearrange("(t p) l -> p t l", p=128))
            for jt in range(NB):
                ssum = spool.tile([128, 1], f32, tag="sum")
                rsum = spool.tile([128, 1], f32, tag="rec")
                ex = spool.tile([128, L], f32, tag="ex")
                nc.scalar.activation(ex[:], lg_sb[:, jt], EXP, accum_out=ssum[:])
                nc.vector.reciprocal(rsum[:], ssum[:])
                hi8 = qT[:, jt, 0:L].bitcast(f8)
                nc.vector.tensor_scalar_mul(hi8, ex[:], rsum[:])
                d = spool.tile([128, L], f32, tag="d")
                nc.vector.scalar_tensor_tensor(
                    d[:], ex[:], rsum[:], hi8, op0=MUL, op1=SUB
                )
                nc.vector.tensor_scalar_mul(
                    qT[:, jt, LO : LO + L].bitcast(f8), d[:], 64.0
                )

            def filt(t, ss0, ss1, sb0, sb1, first, last):
                # 4 DoubleRow MMs per pair; h-halves in separate PSUM tiles
                lhs = qT[:, 2 * t : 2 * t + 2, 0:LW].bitcast(f8)
                for h, (sps, sbs) in enumerate(((ss0, sb0), (ss1, sb1))):
                    hs = slice(h * 512, (h + 1) * 512)
                    nc.tensor.matmul(
                        sps[:], lhs, ksp_st[:, 2 * t : 2 * t + 2, hs].bitcast(f8),
                        start=first, stop=last, perf_mode=DR,
                    )
                    nc.tensor.matmul(
                        sbs[:], lhs, kbi_st[:, 2 * t : 2 * t + 2, hs].bitcast(f8),
                        start=first, stop=last, perf_mode=DR,
                    )

            # ---- build both kernels once (PE gram; DVE byte-encode for sp,
            # ACT exp for bi), iter-0 filter interleaved pair by pair ----
            s_ps = [s_pool.tile([LW, 512], f32, tag=f"s{i}", name=f"s{i}")
                    for i in range(4)]
            for jt in range(NB):
                us = upool.tile([FSP, 128], bf16, tag="usp")
                nc.sync.dma_start(us[:], usp[:, jt * 128 : (jt + 1) * 128])
                ub = upool.tile([FBI, 128], bf16, tag="ubi")
                nc.sync.dma_start(ub[:], ubi[:, jt * 128 : (jt + 1) * 128])
                g = gpool.tile([128, 1024], f32, tag="g")
                for h in range(2):
                    hs = slice(h * 512, (h + 1) * 512)
                    nc.tensor.matmul(
                        g[:, hs], us[:], vsp_sb[:, hs], start=True, stop=True
                    )
                nc.vector.tensor_scalar(
                    ksp_st[:, jt, :], g[:],
                    sqjd_sb[:, jt : jt + 1], BSCALE, op0=ADD, op1=MUL,
                )
                g2 = gpool.tile([128, 1024], f32, tag="g")
                for h in range(2):
                    hs = slice(h * 512, (h + 1) * 512)
                    nc.tensor.matmul(
                        g2[:, hs], ub[:], vbi_sb[:, hs], start=True, stop=True
                    )
                nc.scalar.activation(
                    kbi_st[:, jt, :].bitcast(f8), g2[:], EXP,
                    bias=sqja_sb[:, jt : jt + 1],
                )
                if jt % 2 == 1:
                    t = jt // 2
                    filt(t, *s_ps, t == 0, t == NP - 1)

            for step in range(num_iters):
                if step > 0:
                    # h0 pass then h1 pass, pairs in chunk order: the label
                    # phase for i-tiles 0-3 overlaps the h1 pass
                    s_ps = [s_pool.tile([LW, 512], f32, tag=f"s{i}",
                                        name=f"s{i}") for i in range(4)]
                    order = [GC * k + c for c in range(GC)
                             for k in range(NUM_CORES)]
                    for h in range(2):
                        for idx, t in enumerate(order):
                            first, last = idx == 0, idx == len(order) - 1
                            lhs = qT[:, 2 * t : 2 * t + 2, 0:LW].bitcast(f8)
                            hs = slice(h * 512, (h + 1) * 512)
                            nc.tensor.matmul(
                                s_ps[h][:], lhs,
                                ksp_st[:, 2 * t : 2 * t + 2, hs].bitcast(f8),
                                start=first, stop=last, perf_mode=DR,
                            )
                            nc.tensor.matmul(
                                s_ps[2 + h][:], lhs,
                                kbi_st[:, 2 * t : 2 * t + 2, hs].bitcast(f8),
                                start=first, stop=last, perf_mode=DR,
                            )

                last_it = step == num_iters - 1
                for half in range(2):
                    ssp_sb = wpool.tile([LW, 512], f32, tag="ssb")
                    sbi_sb = wpool.tile([LW, 512], f32, tag="ssb")
                    nc.vector.tensor_copy(ssp_sb[:], s_ps[half][:])
                    nc.vector.tensor_copy(sbi_sb[:], s_ps[2 + half][:])
                    for it in range(4 * half, 4 * half + 4):
                        msp = gpool.tile([128, L + 1], f32, tag="g", name="msp")
                        mbi = gpool.tile([128, L + 1], f32, tag="g", name="mbi")
                        lo, hi = (it - 4 * half) * 128, (it - 4 * half + 1) * 128
                        glo, ghi = it * 128, (it + 1) * 128
                        # fused transpose + label matmul; limb unscale and
                        # normalizer column are folded into the w-stack
                        nc.tensor.matmul(
                            msp[:], ssp_sb[:, lo:hi], wsp_sb[:],
                            start=True, stop=True,
                        )
                        nc.tensor.matmul(
                            mbi[:], sbi_sb[:, lo:hi], wbi_sb[:],
                            start=True, stop=True,
                        )
                        rsp = spool.tile([128, 1], f32, tag="rn")
                        rbi = spool.tile([128, 1], f32, tag="rn")
                        nc.vector.reciprocal(rsp[:], msp[:, L : L + 1])
                        nc.vector.reciprocal(rbi[:], mbi[:, L : L + 1])
                        tmp = spool.tile([128, L], f32, tag="tmp")
                        cur = spool.tile([128, L], f32, tag="cur")
                        nc.vector.scalar_tensor_tensor(
                            tmp[:], msp[:, 0:L], rsp[:], unt_sb[:, it],
                            op0=MUL, op1=ADD,
                        )
                        nc.vector.scalar_tensor_tensor(
                            cur[:], mbi[:, 0:L], rbi[:], tmp[:],
                            op0=MUL, op1=ADD,
                        )
                        ex = spool.tile([128, L], f32, tag="ex")
                        ssum = spool.tile([128, 1], f32, tag="sum")
                        rsum = spool.tile([128, 1], f32, tag="rec")
                        nc.scalar.activation(
                            ex[:], cur[:], EXP, accum_out=ssum[:]
                        )
                        nc.vector.reciprocal(rsum[:], ssum[:])
                        if last_it:
                            nc.vector.tensor_scalar_mul(ex[:], ex[:], rsum[:])
                            nc.sync.dma_start(outq[glo:ghi, :], ex[:])
                        else:
                            hi8 = stg[:, it, 0:L].bitcast(f8)
                            nc.vector.tensor_scalar_mul(hi8, ex[:], rsum[:])
                            d = spool.tile([128, L], f32, tag="d")
                            nc.vector.scalar_tensor_tensor(
                                d[:], ex[:], rsum[:], hi8, op0=MUL, op1=SUB
                            )
                            nc.vector.tensor_scalar_mul(
                                stg[:, it, LO : LO + L].bitcast(f8), d[:], 64.0
                            )
                            nc.sync.dma_start(qsl[glo:ghi, :], stg[:, it, :])
                            if it % 2 == 1:
                                # chunk c = rows [c*256,(c+1)*256) of every
                                # core's slice: gather + one scatter into qT
                                c = it // 2
                                cs = R // GC
                                nc.gpsimd.collective_compute(
                                    "AllGather",
                                    mybir.AluOpType.bypass,
                                    replica_groups=[list(range(NUM_CORES))],
                                    ins=[qsl[c * cs : (c + 1) * cs, :].opt()],
                                    outs=[qfull[c].opt()],
                                )
                                for k in range(NUM_CORES):
                                    jt0 = k * RT + 2 * c
                                    nc.gpsimd.dma_start(
                                        qT[:, jt0 : jt0 + 2, :],
                                        qfull[c, k].rearrange(
                                            "(tt p) x -> p tt x", p=128
                                        ),
                                    )

    nc.compile()
    _CACHE[key] = nc
    return nc


def _host_inputs(image, logits, unary, spatial_ker_weights, bilateral_ker_weights,
                 compatibility_matrix):
    img = np.asarray(image, np.float32)[0].reshape(3, N)
    zz, yy, xx = np.meshgrid(
        np.arange(D), np.arange(H), np.arange(W), indexing="ij"
    )
    pos = np.stack([zz, yy, xx]).reshape(3, N).astype(np.float32)

    import ml_dtypes

    def to_bf16(x):
        return x.astype(ml_dtypes.bfloat16).astype(np.float32)

    def uv(feats):
        # two-limb bf16 split: f = hi + lo (+ dropped 2^-16 residual)
        fh = to_bf16(feats)
        fl = to_bf16(feats - fh)
        ft = fh + fl                     # the features the device actually uses
        sq = (ft * ft).sum(0, dtype=np.float64).astype(np.float32)
        sh = to_bf16(-0.5 * sq)          # i-side sq limbs; truncation cancels in S/n
        sl = to_bf16(-0.5 * sq - sh)
        ones = np.ones((1, N), np.float32)
        u = np.concatenate([fh, fh, fl, fl, ones, ones], 0)
        v = np.concatenate([fh, fl, fh, fl, sh[None], sl[None]], 0)
        bf = ml_dtypes.bfloat16
        return (np.ascontiguousarray(u).astype(bf),
                np.ascontiguousarray(v).astype(bf), sq)

    u_sp, v_sp, sq_sp_ = uv(pos / GAMMA)
    u_bi, v_bi, sq_bi_ = uv(np.concatenate([pos / ALPHA, img / BETA], 0))
    # exact fp32 j-side bias, laid out (partition, jtile)
    sqjd_np = np.ascontiguousarray(
        (-0.5 * sq_sp_ + BOFF).reshape(NB, 128).T
    )
    sqja_np = np.ascontiguousarray((-0.5 * sq_bi_).reshape(NB, 128).T)

    cm = np.asarray(compatibility_matrix, np.float32)
    wa_t = (cm @ np.asarray(spatial_ker_weights, np.float32)).T
    wb_t = (cm @ np.asarray(bilateral_ker_weights, np.float32)).T

    def wstack(wt):
        # [W; zeros; W/64] with indicator column 21 -> routes S[21]=n to
        # output column 21
        w = np.zeros((LW, L + 1), np.float32)
        w[0:L, 0:L] = wt
        w[LO : LO + L, 0:L] = wt / 64.0
        w[21, L] = 1.0
        return np.ascontiguousarray(w)

    un_t = np.ascontiguousarray(np.asarray(unary, np.float32)[0].reshape(L, N).T)
    lg_t = np.ascontiguousarray(np.asarray(logits, np.float32)[0].reshape(L, N).T)

    maps = []
    for c in range(NUM_CORES):
        cols = slice(c * R, (c + 1) * R)
        maps.append({
            "usp": u_sp,
            "vsp": np.ascontiguousarray(v_sp[:, cols]),
            "ubi": u_bi,
            "vbi": np.ascontiguousarray(v_bi[:, cols]),
            "unt": np.ascontiguousarray(un_t[cols]),
            "lgt": lg_t,
            "wsp": wstack(wa_t),
            "wbi": wstack(wb_t),
            "sqjd": sqjd_np,
            "sqja": sqja_np,
        })
    return maps


def kernel(**inputs):
    from concourse.bass_utils import run_bass_kernel_spmd

    nc = _build()
    in_maps = _host_inputs(**inputs)
    res = run_bass_kernel_spmd(nc, in_maps, core_ids=list(range(NUM_CORES)))
    full = np.concatenate([res.results[c]["outq"] for c in range(NUM_CORES)], 0)
    return np.ascontiguousarray(full.T).reshape(1, L, D, H, W).astype(np.float32)
